# revision 21
# baseline (speedup 1.0000x reference)
"""Trainium2 Bass kernel for 16-head MultiHeadAttention (B=2, S=2048, D=1024, f32).

Sharding: 8 cores = 2 (batch) x 4 (head groups of 4 heads).
Each core gets a col-shard of Wq/Wk/Wv ([1024,256]) + row-shard of Wo ([256,1024]),
computes a full [2048,1024] partial output split across two DRAM tensors
(one per head-pair); the host sums the 16 partials into [2,2048,1024].

On-device pipeline (transposed layouts, seq on the free axis):
  QT/KT = Wpair^T @ x^T             -> [128(=2 heads x 64), 2048] f32r
  VT    = Wv_pair^T @ xv^T, then PE-transposed to V_aug [j, head, 65]
          (65th column = ones so AV emits softmax denominators for free)
  sT    = KT_h^T-slice @ QT_h-slice, two heads row-packed in the PE array
          concurrently via tile_position (0,0)/(64,0)
  expT  = exp(0.125 * sT) via ACT from PSUM [128,1024] spans -> f32r
  outT/rowsum = V_aug^T @ expT          (M=65: rows 0-63 outT, row 64 rowsum)
  per-chunk: rowsum row -> DMA partition-gather -> lane-parallel reciprocal
          -> K=1 ones-matmul broadcast -> multiplied into outT (f32r)
  partial = outT^T @ Wo_shard           (K=128 over stacked head pairs),
          emitted per 256-row group as soon as its outT columns are ready.

All matmuls run as float32r (TF32-like 11-bit mantissa): full PE speed at
near-fp32 accuracy. Host pre-rounds inputs to the fp32r grid.
"""

import sys

import numpy as np

if "/opt/trn_rl_repo" not in sys.path:
    sys.path.insert(0, "/opt/trn_rl_repo")

import concourse.bacc as bacc
import concourse.mybir as mybir
import concourse.tile as tile
from concourse.masks import make_identity

F32 = mybir.dt.float32
F32R = mybir.dt.float32r

B, S, D, H = 2, 2048, 1024, 16
DK = D // H          # 64
HL = 4               # heads per core
DG = HL * DK         # 256
SCALE = 0.125        # 1/sqrt(DK)

ET = D // 128        # 8 e-tiles
JT = S // 128        # 16 j-tiles
QC = S // 512        # 4 q-chunks


def _round_fp32r(x: np.ndarray) -> np.ndarray:
    """Round f32 to the fp32r grid (11-bit mantissa, RNE), like walrus fp32_to_fp32r."""
    u = x.view(np.uint32).astype(np.uint64)
    u = (u + 0x7FF + ((u >> 12) & 1)) & np.uint64(0xFFFFF000)
    return u.astype(np.uint32).view(np.float32)


def _build_nc():
    nc = bacc.Bacc("TRN2", target_bir_lowering=False, debug=False)

    xq = nc.dram_tensor("xq", [D, S], F32, kind="ExternalInput").ap()
    xk = nc.dram_tensor("xk", [D, S], F32, kind="ExternalInput").ap()
    xv = nc.dram_tensor("xv", [D, S], F32, kind="ExternalInput").ap()
    wq = nc.dram_tensor("wq", [D, DG], F32, kind="ExternalInput").ap()
    wk = nc.dram_tensor("wk", [D, DG], F32, kind="ExternalInput").ap()
    wv = nc.dram_tensor("wv", [D, DG], F32, kind="ExternalInput").ap()
    wo = nc.dram_tensor("wo", [DG, D], F32, kind="ExternalInput").ap()
    out = nc.dram_tensor("out", [S, D], F32, kind="ExternalOutput").ap()
    out2 = nc.dram_tensor("out2", [S, D], F32, kind="ExternalOutput").ap()

    with tile.TileContext(nc) as tc:
        with (
            tc.tile_pool(name="wpool", bufs=1) as wpool,
            tc.tile_pool(name="xin", bufs=3) as xin,
            tc.tile_pool(name="proj", bufs=1) as proj,
            tc.tile_pool(name="expp", bufs=3) as expp,
            tc.tile_pool(name="stp", bufs=5) as stp,
            tc.tile_pool(name="work", bufs=2) as work,
            tc.tile_pool(name="small", bufs=2) as small,
        ):
            # ---- constants + early weights (wk/wq on the fast queues) ------
            wk_sb = [wpool.tile([128, DG], F32R, tag=f"wk{e}", name=f"wk{e}")
                     for e in range(ET)]
            wq_sb = [wpool.tile([128, DG], F32R, tag=f"wq{e}", name=f"wq{e}")
                     for e in range(ET)]
            wv_sb = [wpool.tile([128, DG], F32R, tag=f"wv{e}", name=f"wv{e}")
                     for e in range(ET)]

            ones16 = wpool.tile([16, 64], F32, tag="ones16", name="ones16")
            nc.vector.memset(ones16, 1.0)
            ones_r = wpool.tile([16, 64], F32R, tag="ones_r", name="ones_r")
            nc.vector.tensor_copy(ones_r, ones16)
            ones_col = wpool.tile([128, 64], F32, tag="ones_col", name="ones_col")
            nc.vector.memset(ones_col, 1.0)
            ident_f = wpool.tile([128, 128], F32, tag="ident_f", name="ident_f")
            make_identity(nc, ident_f)
            ident = wpool.tile([128, 128], F32R, tag="ident", name="ident")
            nc.vector.tensor_copy(ident, ident_f)

            # ---- persistent activation tiles -------------------------------
            kt_sb = [proj.tile([128, S], F32R, tag=f"kt{p}", name=f"kt{p}")
                     for p in range(2)]
            qt_sb = [proj.tile([128, S], F32R, tag=f"qt{p}", name=f"qt{p}")
                     for p in range(2)]
            v_sb = [proj.tile([128, JT // 2, HL, DK + 1], F32R,
                              tag=f"v{hh}", name=f"v{hh}") for hh in range(2)]

            # ---- phase A: projections (8 PSUM accumulators, x streamed) ----
            with tc.tile_pool(name="ps_a", bufs=1, space="PSUM") as ps_a:
                QS = (nc.sync, nc.scalar, nc.gpsimd)

                def proj_pairs(x_dram, w_tiles, dst_tiles, nm, w_dram):
                    accs = [ps_a.tile([128, 512], F32, tag="pa", bufs=8,
                                      name=f"acc_{nm}_{p}_{c}")
                            for p in range(2) for c in range(QC)]
                    # DMA emission order = per-queue service order: lead with
                    # the first x tiles, then the (small) weight tiles, then
                    # the remaining x tiles, round-robin across 3 queues.
                    xt, rr = [None] * ET, 0

                    def load_x(e):
                        nonlocal rr
                        xt[e] = xin.tile([128, S], F32R, tag="xs", name=f"x_{nm}{e}")
                        for hf in range(2):
                            QS[rr % 3].dma_start(
                                xt[e][:, hf * 1024:(hf + 1) * 1024],
                                x_dram.bitcast(F32R)[
                                    e * 128:(e + 1) * 128,
                                    hf * 1024:(hf + 1) * 1024],
                            )
                            rr += 1

                    for e in range(3):
                        load_x(e)
                    for e in range(ET):
                        QS[(rr + e) % 3].dma_start(
                            w_tiles[e], w_dram.bitcast(F32R)[e * 128:(e + 1) * 128, :])
                    for e in range(3, ET):
                        load_x(e)
                    for e in range(ET):
                        for p in range(2):
                            for c in range(QC):
                                nc.tensor.matmul(
                                    accs[p * QC + c],
                                    w_tiles[e][:, p * 128:(p + 1) * 128],
                                    xt[e][:, c * 512:(c + 1) * 512],
                                    start=(e == 0), stop=(e == ET - 1),
                                )
                    for p in range(2):
                        for c in range(QC):
                            nc.vector.tensor_copy(
                                dst_tiles[p][:, c * 512:(c + 1) * 512],
                                accs[p * QC + c],
                            )

                proj_pairs(xk, wk_sb, kt_sb, "k", wk)
                proj_pairs(xq, wq_sb, qt_sb, "q", wq)
                vt_sb = [proj.tile([128, S], F32R, tag=f"ot{p}", name=f"vt{p}")
                         for p in range(2)]
                proj_pairs(xv, wv_sb, vt_sb, "v", wv)

            # wo needed from mid-phase-B on; load during phase A tail
            wo_sb = [wpool.tile([128, D], F32R, tag=f"wo{p}", name=f"wo{p}")
                     for p in range(2)]
            for p in range(2):
                nc.gpsimd.dma_start(wo_sb[p], wo.bitcast(F32R)[p * 128:(p + 1) * 128, :])

            # V_aug via PE transpose of VT
            with tc.tile_pool(name="ps_t", bufs=2, space="PSUM") as ps_t:
                for p in range(2):
                    for jt in range(JT):
                        pt = ps_t.tile([128, 128], F32R, tag="pt", name=f"pt{p}_{jt}")
                        nc.tensor.transpose(
                            pt, vt_sb[p][:, jt * 128:(jt + 1) * 128], ident)
                        hh, j2 = divmod(jt, JT // 2)
                        nc.vector.tensor_copy(
                            v_sb[hh][:, j2, 2 * p:2 * p + 2, 0:DK],
                            pt.rearrange("j (h d) -> j h d", h=2),
                        )
                for hh in range(2):
                    nc.vector.tensor_copy(
                        v_sb[hh][:, :, :, DK:DK + 1],
                        ones_col[:, 0:32].rearrange("p (a b) -> p a b", a=8)[:, :, :, None],
                    )

            # ---- phases B+C: attention + output projection -----------------
            outt_sb = [proj.tile([128, S], F32R, tag=f"ot{p}", name=f"outt{p}")
                       for p in range(2)]

            with (
                tc.tile_pool(name="ps_sc", bufs=2, space="PSUM") as ps_sc,
                tc.tile_pool(name="ps_av", bufs=2, space="PSUM") as ps_av,
                tc.tile_pool(name="ps_wo", bufs=2, space="PSUM") as ps_wo,
            ):
                def wo_block(p, qg):
                    # partial += outT_p^T @ Wo_p for q-tiles 2qg, 2qg+1
                    osb = work.tile([128, 2, 1024], F32, tag="osb",
                                    name=f"osb{p}_{qg}")
                    for qq in range(2):
                        qt = qg * 2 + qq
                        for ch in range(2):
                            acc = ps_wo.tile([128, 512], F32, tag="po",
                                             name=f"po{p}_{qt}_{ch}")
                            nc.tensor.matmul(
                                acc,
                                outt_sb[p][:, qt * 128:(qt + 1) * 128],
                                wo_sb[p][:, ch * 512:(ch + 1) * 512],
                                start=True, stop=True,
                            )
                            nc.vector.tensor_copy(
                                osb[:, qq, ch * 512:(ch + 1) * 512], acc)
                    dst = (out if p == 0 else out2)[
                        qg * 256:(qg + 1) * 256, :].rearrange(
                        "(a j) e -> j a e", a=2)
                    nc.sync.dma_start(dst, osb)

                pending_norm = []
                for p in range(2):
                    hA, hB = 2 * p, 2 * p + 1
                    for c in range(QC):
                        csl = slice(c * 512, (c + 1) * 512)
                        # scores + exp, row-packed head pairs, 1 jt per group
                        exp_tiles = []
                        for jt in range(JT):
                            jsl = slice(jt * 128, (jt + 1) * 128)
                            sc = ps_sc.tile([128, 2 * 512], F32, tag="sc",
                                            name=f"sc{p}_{c}_{jt}")
                            nc.tensor.matmul(
                                sc[:, 0:512],
                                kt_sb[p][0:64, jsl],
                                qt_sb[p][0:64, csl],
                                start=True, stop=True,
                                tile_position=(0, 0),
                            )
                            nc.tensor.matmul(
                                sc[:, 512:1024],
                                kt_sb[p][64:128, jsl],
                                qt_sb[p][64:128, csl],
                                start=True, stop=True,
                                tile_position=(64, 0),
                            )
                            ex = expp.tile([128, 2, 512], F32R, tag="ex",
                                           name=f"ex{p}_{c}_{jt}")
                            nc.scalar.activation(
                                out=ex,
                                in_=sc.rearrange("j (t q) -> j t q", t=2),
                                func=mybir.ActivationFunctionType.Exp,
                                scale=SCALE,
                            )
                            exp_tiles.append(ex)
                        # deferred from previous chunk: normalization
                        # matmuls + Wo (the serial recip chain has finished
                        # by now, so the in-order PE stream won't stall)
                        for fn in pending_norm:
                            fn()
                        pending_norm = []
                        # AV for both heads (full-row K=128 accumulation)
                        avA = ps_av.tile([128, 512], F32, tag="av", name=f"avA{p}_{c}")
                        avB = ps_av.tile([128, 512], F32, tag="av", name=f"avB{p}_{c}")
                        for jt in range(JT):
                            hh, j2 = divmod(jt, JT // 2)
                            ex = exp_tiles[jt]
                            nc.tensor.matmul(
                                avA[0:DK + 1, :],
                                v_sb[hh][:, j2, hA, :],
                                ex[:, 0, :],
                                start=(jt == 0), stop=(jt == JT - 1),
                            )
                            nc.tensor.matmul(
                                avB[0:DK + 1, :],
                                v_sb[hh][:, j2, hB, :],
                                ex[:, 1, :],
                                start=(jt == 0), stop=(jt == JT - 1),
                            )
                        # stage outT + rowsum, run the recip chain now
                        # (DVE/DMA work, overlaps the next chunk's scores)
                        for i, av in ((0, avA), (1, avB)):
                            st = stp.tile([128, 512], F32, tag="st",
                                          name=f"st{p}_{c}_{i}")
                            nc.vector.tensor_copy(st[0:DK + 1, :], av[0:DK + 1, :])
                            rsg = small.tile([4, 128], F32, tag=f"rsg{i}",
                                             name=f"rsg{p}_{c}_{i}")
                            nc.sync.dma_start(
                                rsg,
                                st[DK:DK + 1, :].rearrange(
                                    "one (pp f) -> one pp f", pp=4),
                            )
                            nc.vector.reciprocal(rsg, rsg)
                            rrow = small.tile([1, 512], F32, tag=f"rrow{i}",
                                              name=f"rrow{p}_{c}_{i}")
                            nc.sync.dma_start(
                                rrow.rearrange("one (pp f) -> one pp f", pp=4),
                                rsg,
                            )
                            rbc = small.tile([64, 512], F32, tag=f"rbc{i}",
                                             name=f"rbc{p}_{c}_{i}")
                            nc.gpsimd.partition_broadcast(rbc, rrow)
                            nc.vector.tensor_tensor(
                                outt_sb[p][slice(i * 64, (i + 1) * 64), csl],
                                st[0:DK, :],
                                rbc,
                                mybir.AluOpType.mult,
                            )

                        def norm_and_wo(p=p, c=c):
                            wo_block(p, 2 * c)
                            wo_block(p, 2 * c + 1)

                        pending_norm = [norm_and_wo]

                for fn in pending_norm:
                    fn()

    nc.compile()
    return nc


_NC = None


def _get_nc():
    global _NC
    if _NC is None:
        _NC = _build_nc()
    return _NC


def make_in_maps(query, key, value, Wq, Wk, Wv, Wo):
    query = _round_fp32r(np.ascontiguousarray(query, dtype=np.float32))
    key_ = _round_fp32r(np.ascontiguousarray(key, dtype=np.float32))
    value = _round_fp32r(np.ascontiguousarray(value, dtype=np.float32))
    Wq = _round_fp32r(np.ascontiguousarray(Wq, dtype=np.float32))
    Wk = _round_fp32r(np.ascontiguousarray(Wk, dtype=np.float32))
    Wv = _round_fp32r(np.ascontiguousarray(Wv, dtype=np.float32))
    Wo = _round_fp32r(np.ascontiguousarray(Wo, dtype=np.float32))

    xqT = [np.ascontiguousarray(query[b].T) for b in range(B)]
    xkT = [np.ascontiguousarray(key_[b].T) for b in range(B)]
    xvT = [np.ascontiguousarray(value[b].T) for b in range(B)]

    in_maps = []
    for core in range(8):
        b, g = divmod(core, 4)
        sl = slice(g * DG, (g + 1) * DG)
        in_maps.append({
            "xq": xqT[b],
            "xk": xkT[b],
            "xv": xvT[b],
            "wq": np.ascontiguousarray(Wq[:, sl]),
            "wk": np.ascontiguousarray(Wk[:, sl]),
            "wv": np.ascontiguousarray(Wv[:, sl]),
            "wo": np.ascontiguousarray(Wo[sl, :]),
        })
    return in_maps


def combine_results(results):
    out = np.zeros((B, S, D), dtype=np.float32)
    for core in range(8):
        out[core // 4] += results[core]["out"]
        out[core // 4] += results[core]["out2"]
    return out


def kernel(query, key, value, Wq, Wk, Wv, Wo, _trace=False):
    from concourse import bass_utils

    nc = _get_nc()
    in_maps = make_in_maps(query, key, value, Wq, Wk, Wv, Wo)
    r = bass_utils.run_bass_kernel_spmd(
        nc, in_maps, core_ids=list(range(8)), trace=_trace
    )
    kernel.last_results = r
    return combine_results(r.results)
